# revision 3
# baseline (speedup 1.0000x reference)
"""AttentionBlock for Trainium2: row-tiled PE, dual-engine exp, fp8 AV.

Data-parallel over batch: each of the 8 NeuronCores runs one sample
end-to-end (no cross-core communication). Per-core pipeline (v2):

  - x loaded once as bf16 via casting DMAs (both duplicated SBUF halves);
    GroupNorm stats computed from the bf16 copy; the affine h = a*x + b is
    folded into the projection weights/biases.
  - Q IS ELIMINATED: S = hx^T kk with hx = a*x (one DVE 4x-mode op) and
    kk = (Wg*a)^T x + w, Wg = (Wq^T Wk)/8 host-side, w = Wk^T bq_eff
    folded as a free per-partition bias into the ACT evacuation of kk.
    The m-row bias term sum_j bq_eff[j] k[j,m] is exactly x^T (a*w).
  - Wp IS FOLDED INTO V: v2 = (Wp Wv * a) x, so AV directly produces the
    projected output; the 65th vT ones-column accumulates the softmax
    denominator. Post per n-tile: ACT evacs the den row, a 1-row PE
    matmul broadcasts it to partitions 0:64, DVE does approx-reciprocal
    + a fused (av*recip + bpp) custom op; the +x residual rides the
    output DMA (accum onto the x-prefilled out buffer).
  - Projections and scores run in 64x128 row-tiled mode: two independent
    64-contraction tiles (SBUF partitions 0:64 / 64:128) execute
    concurrently. Score pairs land in rotating 2-bank PSUM pair-slots.
  - exp of the 16.7M scores is the roofline. Work is split across TWO
    engines by pairs:
      * ScalarE: exact exp, [128,1024] PSUM->SBUF fp8e4 (~1 elem/lane/cyc).
      * VectorE: Schraudolph bit-trick exp in ONE tensor_scalar op:
        int8(11.54*s + 55.5) reinterpreted as fp8e4 == exp(s)*(1 +- 7%).
  - AV runs in fp8 DoubleRow mode: one matmul per pair contracts 256
    into a single [65,512] accumulator. AV for tile nt runs one n-tile
    lagged in half-blocks; the last n-tile drains in quarter-blocks.
  - PSUM: 3 pair-slots (6 banks) + av x2 (2 banks) = 8.
"""

import numpy as np
import ml_dtypes

import concourse.bacc as bacc
import concourse.mybir as mybir
import concourse.dve_ops as dve_ops
from concourse.dve_spec import Spec, Src0, Src1, C0, lower
from concourse.dve_uop import DveOpSpec
from concourse.tile import TileContext
from concourse.bass_utils import run_bass_kernel_spmd


def _get_muladd():
    """out = in0*in1 + s0 as one DVE op (registered via the documented
    OPS-append extension path; sha pinned programmatically)."""
    for op in dve_ops.OPS:
        if op.name == "ATT_MULADD":
            return op
    spec = Spec(body=Src0 * Src1 + C0,
                reference=lambda in0, in1, s0, s1, imm2: in0 * in1 + s0)
    row = dve_ops._CUSTOM_DVE_ROW_BASE + len(dve_ops.OPS)
    shas = {}
    for ver in ("v3", "v4"):
        shas[ver] = DveOpSpec(name="ATT_MULADD", opcode=row,
                              uops=lower(spec, ver=ver), rd1_en=True).sha(ver)
    op = dve_ops.DveOp("ATT_MULADD", spec, subdim=False, uops_sha=shas)
    dve_ops.OPS.append(op)
    dve_ops.CUSTOM_DVE_SPECS[op.name] = spec
    dve_ops._SUB_OPCODE_FOR_NAME[op.name] = row
    return op


MULADD = _get_muladd()

FP = mybir.dt.float32
F16 = mybir.dt.bfloat16
I16 = mybir.dt.int16
F8 = mybir.dt.float8e4
I8 = mybir.dt.int8
B, C, H, W = 8, 64, 64, 64
N = H * W            # 4096
G = 8
NT = 512             # n-tile width
N_NT = N // NT       # 8
NPAIR = 16           # score pairs (of 2x128 m-rows) per n-tile
EPS = 1e-5
COPY = mybir.ActivationFunctionType.Copy
IDENT = mybir.ActivationFunctionType.Identity
EXP = mybir.ActivationFunctionType.Exp
ADD = mybir.AluOpType.add
MUL = mybir.AluOpType.mult
SUB = mybir.AluOpType.subtract

# Schraudolph constants for fp8e4 target (2^3 mantissa scale); scores are
# in [-2.84, 2.84] for this problem so exp fits fp8e4 with no shift and the
# int8 bits stay in [20, 90].
A_SCH = 11.5415603
B_SCH = 55.5

# which pairs each n-tile sends to the DVE trick-exp (rest go to ACT).
# Early n-tiles lighter on DVE (it carries kk/vT evacuations there).
_D7 = {2, 4, 7, 9, 11, 13, 15}
_D8 = {1, 3, 5, 7, 9, 11, 13, 15}
DVE_MAP = {0: {4, 9, 13}, 1: {2, 5, 8, 11, 14}, 2: _D7, 3: _D8,
           4: _D7, 5: _D8, 6: _D8, 7: _D8}

last_run_info = {}


class OneActSetBacc(bacc.Bacc):
    """Force every ACT table load to set 6 (natural_log_exp_and_others:
    exp/ln/square/copy) and drop redundant reloads."""

    NL_EXP_SET = 6

    def insert_act_table_loads(self):
        super().insert_act_table_loads()
        for blk in self.main_func.blocks:
            keep = []
            seen = False
            for ins in blk.instructions:
                if isinstance(ins, mybir.InstLoadActFuncSet):
                    ins.act_func_set_id = self.NL_EXP_SET
                    si = ins.sync_info
                    clean = si is None or (not si.on_wait and not si.on_update)
                    if seen and clean:
                        continue
                    seen = True
                keep.append(ins)
            if len(keep) != len(blk.instructions):
                blk.instructions[:] = keep


def build_program():
    nc = OneActSetBacc()

    x_d = nc.dram_tensor("x", [C, N], FP, kind="ExternalInput")
    # cf32 [128, 141]: 0 bq2(=bq/8) | 1 bpp | 2 gamma2 | 3 beta2 | 4:12 gmask
    #                  | 12:140 gbcast2 (rows 0:8) | 140 eps
    cf32_d = nc.dram_tensor("cf32", [128, 141], FP, kind="ExternalInput")
    # cb16 [128, 512]: 0:128 wq64 | 128:256 wg2 ((Wq^T Wk)/8 tiled 2x2)
    #                  | 256:320 wv2 ((Wp Wv)^T tiled 2x1) | 320:448 wkN2
    #                  (Wk natural [j,c] tiled 2x2)
    cb16_d = nc.dram_tensor("cb16", [128, 512], F16, kind="ExternalInput")
    out_d = nc.dram_tensor("out", [C, N], FP, kind="ExternalOutput")

    with TileContext(nc) as tc:
        with (
            tc.tile_pool(name="const", bufs=1) as const,
            tc.tile_pool(name="big", bufs=1) as big,
            tc.tile_pool(name="epool", bufs=22) as epool,
            tc.tile_pool(name="small", bufs=4) as small,
            tc.tile_pool(name="scr", bufs=2) as scrp,
            tc.tile_pool(name="outp", bufs=3) as outp,
            tc.tile_pool(name="sps", bufs=3, space="PSUM") as sps,
            tc.tile_pool(name="avp", bufs=2, space="PSUM") as avp,
        ):
            # ---------------- constants ----------------
            cf32s = small.tile([128, 141], FP, tag="cf32s")
            cb16s = small.tile([128, 512], F16, tag="cb16s")
            nc.sync.dma_start(out=cf32s[:], in_=cf32_d[:])
            nc.sync.dma_start(out=cb16s[:], in_=cb16_d[:])
            cf32 = const.tile([128, 141], FP, tag="cf32")
            cb16 = const.tile([128, 512], F16, tag="cb16")
            nc.vector.tensor_copy(out=cf32[:], in_=cf32s[:])
            nc.vector.tensor_copy(out=cb16[:], in_=cb16s[:])
            bq2 = cf32[:, 0:1]
            bpp = cf32[0:C, 1:2]
            gamma2 = cf32[:, 2:3]
            beta2 = cf32[:, 3:4]
            gmask = cf32[0:C, 4:12]
            gbcast2 = cf32[0:G, 12:140]
            eps_sb = cf32[:, 140:141]
            wq64 = cb16[:, 0:128]
            wg2 = cb16[:, 128:256]
            wv2 = cb16[:, 256:320]
            wkN2 = cb16[:, 320:448]

            ones_col = const.tile([128, C], F16, tag="ones_col")
            nc.vector.memset(ones_col[:], 1.0)

            # ---------------- x16 load (casting DMA, dup halves) + stats ----
            x16 = big.tile([128, N], F16, tag="x16")
            NCH = 4
            CH = N // NCH
            for j in range(NCH):
                sl = slice(j * CH, (j + 1) * CH)
                nc.gpsimd.dma_start(out=x16[0:C, sl], in_=x_d[:, sl])
            nc.gpsimd.dma_start(out=x16[C:128, :], in_=x_d[:, :])
            # prefill out with x so the per-tile output DMA can accumulate
            # the +x residual for free (same SWDGE engine -> ordered before
            # the first accum ~20us later).
            nc.gpsimd.dma_start(out=out_d[:, :], in_=x_d[:, :])

            sums = small.tile([C, NCH, 2], FP, tag="gn_sums")
            for j in range(NCH):
                sl = slice(j * CH, (j + 1) * CH)
                scr = scrp.tile([C, CH], FP, tag="gn_scr")
                nc.scalar.activation(out=scr[:], in_=x16[0:C, sl],
                                     func=mybir.ActivationFunctionType.Square,
                                     accum_out=sums[:, j, 1:2])
                nc.vector.tensor_reduce(op=ADD, out=sums[:, j, 0:1],
                                        in_=x16[0:C, sl], axis=mybir.AxisListType.X)
            nc.vector.tensor_add(out=sums[:, 0:2, :], in0=sums[:, 0:2, :],
                                 in1=sums[:, 2:4, :])
            nc.vector.tensor_add(out=sums[:, 0, :], in0=sums[:, 0, :],
                                 in1=sums[:, 1, :])

            # PSUM slot rotation: 3 independent [128, 1024] pool tiles (2
            # banks each) so the Tile tracker sequences per-slot, not
            # whole-tile.
            slot_ctr = [0]

            def next_slot():
                slot_ctr[0] += 1
                return sps.tile([128, 1024], FP, tag="sp",
                                name=f"sp_{slot_ctr[0]}")

            # group stats: [G, 2] = gmask.T @ sums  (gmask holds 1/(8N))
            gslot = next_slot()
            nc.tensor.matmul(out=gslot[0:G, 0:2], lhsT=gmask, rhs=sums[:, 0, :])
            gs = small.tile([G, 2], FP, tag="gn_gs")
            nc.vector.tensor_copy(out=gs[:], in_=gslot[0:G, 0:2])
            # nvg = mean^2 - E[x^2]; ln(var+eps) = Ln(-1*nvg + eps)
            nvg = small.tile([G, 1], FP, tag="gn_nvg")
            nc.vector.scalar_tensor_tensor(out=nvg[:], in0=gs[:, 0:1],
                                           scalar=gs[:, 0:1], in1=gs[:, 1:2],
                                           op0=MUL, op1=SUB)
            lnv = small.tile([G, 1], FP, tag="gn_lnv")
            nc.scalar.activation(out=lnv[:], in_=nvg[:],
                                 func=mybir.ActivationFunctionType.Ln,
                                 scale=-1.0, bias=eps_sb[0:G, :])
            rhs2 = small.tile([G, 2], FP, tag="gn_rhs2")
            nc.vector.tensor_copy(out=rhs2[:, 0:1], in_=gs[:, 0:1])
            nc.scalar.activation(out=rhs2[:, 1:2], in_=lnv[:], func=EXP,
                                 scale=-0.5)
            pslot = next_slot()
            nc.tensor.matmul(out=pslot[:, 0:2], lhsT=gbcast2, rhs=rhs2[:])
            a_sb = small.tile([128, 1], FP, tag="gn_a")
            nc.vector.tensor_mul(out=a_sb[:], in0=pslot[:, 1:2], in1=gamma2)
            # negb16 = mean*a - beta  (bf16; = -b)
            negb16 = small.tile([128, 1], F16, tag="gn_negb")
            nc.vector.scalar_tensor_tensor(out=negb16[:], in0=pslot[:, 0:1],
                                           scalar=a_sb[:], in1=beta2,
                                           op0=MUL, op1=SUB)
            wg_eff = const.tile([128, 128], F16, tag="wg_eff")
            wv2_eff = const.tile([128, C], F16, tag="wv2_eff")
            nc.vector.tensor_scalar_mul(out=wg_eff[:], in0=wg2, scalar1=a_sb[:])
            nc.vector.tensor_scalar_mul(out=wv2_eff[:], in0=wv2, scalar1=a_sb[:])
            # biases: bq_eff = bq/8 - (Wq(-b))/8 ; bpp_eff = bpp - WpWv(-b)
            bslot = next_slot()
            nc.tensor.matmul(out=bslot[:, 0:1], lhsT=wq64[0:C, :],
                             rhs=negb16[0:C, :])
            nc.tensor.matmul(out=bslot[0:C, 1:2], lhsT=wv2[0:C, :],
                             rhs=negb16[0:C, :])
            bq16 = small.tile([128, 1], F16, tag="bq16")
            bpp_eff = small.tile([C, 1], FP, tag="bpp_eff")
            nc.vector.tensor_sub(out=bq16[:], in0=bq2, in1=bslot[:, 0:1])
            nc.vector.tensor_sub(out=bpp_eff[:], in0=bpp, in1=bslot[0:C, 1:2])
            # w = Wk^T bq_eff (per-partition bias folded into kk evac)
            wslot = next_slot()
            nc.tensor.matmul(out=wslot[:, 0:1], lhsT=wkN2[0:C, :],
                             rhs=bq16[0:C, :])
            w_sb = small.tile([128, 1], FP, tag="w_sb")
            nc.vector.tensor_copy(out=w_sb[:], in_=wslot[:, 0:1])

            # hx = a * x (scores lhsT; carries the m-side GroupNorm scale)
            hx = big.tile([128, N], F16, tag="hx")
            for j in range(NCH):
                sl = slice(j * CH, (j + 1) * CH)
                nc.vector.tensor_scalar_mul(out=hx[:, sl], in0=x16[:, sl],
                                            scalar1=a_sb[:])

            # ---------------- projections (row-tiled pairs) ----------------
            kk2x = big.tile([128, N], F16, tag="kk2x")
            vT = big.tile([128, N // 128, 80], F8, tag="vT")  # 80-byte chunk stride (DoubleRow needs %16==0)
            nc.vector.memset(vT[:, :, C:C + 1], 1.0)

            def emit_kproj(r):
                lo = slice(1024 * r, 1024 * r + 512)
                hi = slice(1024 * r + 512, 1024 * r + 1024)
                s = next_slot()
                nc.tensor.matmul(out=s[:, 0:512], lhsT=wg_eff[0:C, :],
                                 rhs=x16[0:C, lo])
                nc.tensor.matmul(out=s[:, 512:1024], lhsT=wg_eff[C:128, :],
                                 rhs=x16[C:128, hi])
                if r == 0:
                    # split evac so scores pair 0 (kk cols 0:512) starts early
                    nc.scalar.activation(out=kk2x[:, 0:512],
                                         in_=s[:, 0:512], func=IDENT,
                                         bias=w_sb[:])
                    nc.scalar.activation(out=kk2x[:, 512:1024],
                                         in_=s[:, 512:1024], func=IDENT,
                                         bias=w_sb[:])
                else:
                    nc.scalar.activation(out=kk2x[:, 1024 * r:1024 * (r + 1)],
                                         in_=s[:, 0:1024], func=IDENT,
                                         bias=w_sb[:])

            def emit_vproj(g):
                # chunks 8g..8g+7: T0 evens into bank cols 0:256, T8 odds 512:768
                s = next_slot()
                for t in range(4):
                    ce = 8 * g + 2 * t
                    nc.tensor.matmul(out=s[0:128, 64 * t:64 * t + 64],
                                     lhsT=x16[0:C, ce * 128:(ce + 1) * 128],
                                     rhs=wv2_eff[0:C, :])
                    nc.tensor.matmul(out=s[0:128, 512 + 64 * t:512 + 64 * t + 64],
                                     lhsT=x16[C:128, (ce + 1) * 128:(ce + 2) * 128],
                                     rhs=wv2_eff[C:128, :])
                dst = vT[:, 8 * g:8 * g + 8, 0:C].rearrange(
                    "p (t q) c -> p q t c", q=2)
                src = s[:, 0:1024].rearrange("p (q x) -> p q x", q=2)[
                    :, :, 0:256].rearrange("p q (t c) -> p q t c", t=4)
                nc.vector.tensor_copy(out=dst, in_=src)

            # ---------------- attention loop ----------------
            def emit_scores(nt, p):
                nsl = slice(nt * NT, (nt + 1) * NT)
                s = next_slot()
                c0 = 2 * p
                nc.tensor.matmul(out=s[:, 0:512],
                                 lhsT=hx[0:C, c0 * 128:(c0 + 1) * 128],
                                 rhs=kk2x[0:C, nsl], skip_group_check=True)
                nc.tensor.matmul(out=s[:, 512:1024],
                                 lhsT=hx[C:128, (c0 + 1) * 128:(c0 + 2) * 128],
                                 rhs=kk2x[C:128, nsl], skip_group_check=True)
                return s

            def emit_consumer(nt, p, s, dve):
                e = epool.tile([128, 1024], F8, tag="e", name=f"e_{nt}_{p}")
                if dve:
                    nc.vector.tensor_scalar(
                        out=e[:].bitcast(I8), in0=s[:, 0:1024],
                        scalar1=A_SCH, scalar2=B_SCH, op0=MUL, op1=ADD)
                else:
                    nc.scalar.activation(out=e[:], in_=s[:, 0:1024], func=EXP)
                return e

            def emit_av(av, e, p, first, last):
                # fp8 DoubleRow: both 128-row chunks of the pair in one MM
                # (contraction 256 over 128 partitions x 2 interleaved).
                nc.tensor.matmul(out=av[0:C + 1, :],
                                 lhsT=vT[:, 2 * p:2 * p + 2, 0:C + 1],
                                 rhs=e[:].rearrange("p (q n) -> p q n", q=2),
                                 perf_mode=mybir.MatmulPerfMode.DoubleRow,
                                 start=first, stop=last,
                                 skip_group_check=True)

            def emit_post(nt, av):
                # av rows 0:64 already hold the PROJECTED output (Wp folded
                # into v2); row 64 is the softmax denominator. Broadcast den
                # to partitions 0:64 via a 1-row matmul, then
                # out = av*recip(den) + bpp; +x rides the output accum-DMA.
                nsl = slice(nt * NT, (nt + 1) * NT)
                den16 = outp.tile([128, NT], F16, tag="den16")
                nc.scalar.activation(out=den16[C:C + 1, :],
                                     in_=av[C:C + 1, :], func=COPY)
                s = next_slot()
                nc.tensor.matmul(out=s[0:C, 0:512],
                                 lhsT=ones_col[C:C + 1, :],
                                 rhs=den16[C:C + 1, :], skip_group_check=True)
                dbc = outp.tile([C, NT], FP, tag="dbc")
                nc.vector.reciprocal_approx_fast(out=dbc[:], in_=s[0:C, 0:512])
                o_sb = outp.tile([C, NT], FP, tag="o_sb")
                nc.vector._custom_dve(MULADD, out=o_sb[:], in0=av[0:C, :],
                                      in1=dbc[:], s0=bpp_eff[:])
                nc.gpsimd.dma_start(out=out_d[:, nsl], in_=o_sb[:],
                                    accum_op=ADD)

            # startup cascade + steady loop. AV for n-tile nt runs one tile
            # LAGGED, in two half-blocks of 8 DoubleRow MMs, so the PE mode
            # (64x128 scores vs 128x128 DoubleRow) switches only ~4x per nt.
            emit_kproj(0)
            emit_vproj(0)

            CASCADE = {2: [("k", 1)], 4: [("v", 1)],
                       6: [("k", 2)], 8: [("v", 2)],
                       10: [("k", 3)], 12: [("v", 3)]}
            e_tiles = {}
            av_tiles = {}

            def emit_av_block(nt, lo, hi):
                av = av_tiles[nt]
                for p in range(lo, hi):
                    emit_av(av, e_tiles.pop((nt, p)), p,
                            first=(p == 0), last=(p == NPAIR - 1))

            LAST = N_NT - 1
            for nt in range(N_NT):
                dve_set = DVE_MAP[nt]
                av_tiles[nt] = avp.tile([128, NT], FP, tag="av",
                                        name=f"av_{nt}")
                pend = {}
                for p in range(NPAIR):
                    if nt == 0:
                        for kind, r in CASCADE.get(p, []):
                            (emit_kproj if kind == "k" else emit_vproj)(r)
                    pend[p] = emit_scores(nt, p)
                    if p >= 1:
                        e_tiles[(nt, p - 1)] = emit_consumer(
                            nt, p - 1, pend.pop(p - 1), (p - 1) in dve_set)
                    if nt > 0 and p == 7:
                        emit_av_block(nt - 1, 0, 8)
                    if nt > 0 and p == 15:
                        emit_av_block(nt - 1, 8, 16)
                    # last n-tile: drain AV in quarter blocks to shrink the
                    # tail after the final exp
                    if nt == LAST and p in (5, 9, 13):
                        emit_av_block(nt, 4 * ((p - 5) // 4), 4 * ((p - 5) // 4) + 4)
                e_tiles[(nt, NPAIR - 1)] = emit_consumer(
                    nt, NPAIR - 1, pend.pop(NPAIR - 1),
                    (NPAIR - 1) in dve_set)
                if nt > 0:
                    emit_post(nt - 1, av_tiles.pop(nt - 1))
            emit_av_block(LAST, 12, 16)
            emit_post(LAST, av_tiles.pop(LAST))

    nc.finalize()
    return nc


def make_consts(Wq, bq, Wk, Wv, bv, Wp, bp, gn_w, gn_b):
    f32 = np.float32
    gmask = np.zeros((C, G), f32)
    gbcast2 = np.zeros((G, 128), f32)
    for g in range(G):
        gmask[g * 8:(g + 1) * 8, g] = 1.0 / (8.0 * N)
        gbcast2[g, g * 8:(g + 1) * 8] = 1.0
        gbcast2[g, C + g * 8:C + (g + 1) * 8] = 1.0
    Wq_ = np.asarray(Wq, f32)
    Wk_ = np.asarray(Wk, f32)
    Wv_ = np.asarray(Wv, f32)
    Wp_ = np.asarray(Wp, f32)
    WqT = Wq_.T
    cf32 = np.zeros((128, 141), f32)
    cf32[:, 0] = np.tile(np.asarray(bq, f32) / 8.0, 2)
    cf32[0:C, 1] = np.asarray(bp, f32) + Wp_ @ np.asarray(bv, f32)
    cf32[:, 2] = np.tile(np.asarray(gn_w, f32), 2)
    cf32[:, 3] = np.tile(np.asarray(gn_b, f32), 2)
    cf32[0:C, 4:12] = gmask
    cf32[0:G, 12:140] = gbcast2
    cf32[:, 140] = EPS
    cb16 = np.zeros((128, 512), f32)
    cb16[:, 0:128] = np.tile(WqT, (2, 2)) / 8.0
    cb16[:, 128:256] = np.tile((Wq_.T @ Wk_) / 8.0, (2, 2))
    cb16[:, 256:320] = np.tile((Wp_ @ Wv_).T, (2, 1))
    cb16[:, 320:448] = np.tile(Wk_, (2, 2))
    return {
        "cf32": np.ascontiguousarray(cf32),
        "cb16": np.ascontiguousarray(cb16.astype(ml_dtypes.bfloat16)),
    }


_cached = {}


def _install_trace_hook():
    import sys, types
    import antenv
    if "antenv.axon_hooks" in sys.modules:
        return
    mod = types.ModuleType("antenv.axon_hooks")
    holder = {"hook": None}
    mod.set_axon_ntff_profile_hook = lambda h: holder.__setitem__("hook", h)
    mod.get_axon_ntff_profile_hook = lambda: holder["hook"]
    sys.modules["antenv.axon_hooks"] = mod
    antenv.axon_hooks = mod
    from trn_agent_boot.trn_boot import _ntff_profile_via_ctypes
    mod.set_axon_ntff_profile_hook(_ntff_profile_via_ctypes("/opt/axon/libaxon_pjrt.so"))
    import concourse.bass_utils as bu
    bu.upload_artifacts = lambda tmpdir: tmpdir


def kernel(x, gn_w, gn_b, Wq, bq, Wk, bk, Wv, bv, Wp, bp, _trace=False):
    x = np.ascontiguousarray(np.asarray(x, np.float32)).reshape(B, C, N)
    consts = make_consts(Wq, bq, Wk, Wv, bv, Wp, bp, gn_w, gn_b)

    if _trace:
        _install_trace_hook()

    if "nc" not in _cached:
        _cached["nc"] = build_program()
    nc = _cached["nc"]

    in_maps = [dict(consts, x=np.ascontiguousarray(x[i])) for i in range(B)]
    res = run_bass_kernel_spmd(nc, in_maps, core_ids=list(range(B)), trace=_trace)
    last_run_info["exec_time_ns"] = res.exec_time_ns
    last_run_info["mean_exec_time_ns"] = res.mean_exec_time_ns
    out = np.stack([res.results[i]["out"] for i in range(B)], axis=0)
    return out.reshape(B, C, H, W)


# revision 4
# speedup vs baseline: 1.1669x; 1.1669x over previous
"""AttentionBlock for Trainium2: row-tiled PE, dual-engine exp, fp8 AV.

Data-parallel over batch: each of the 8 NeuronCores runs one sample
end-to-end (no cross-core communication). Per-core pipeline (v2):

  - x loaded once as bf16 via casting DMAs (both duplicated SBUF halves);
    GroupNorm stats computed from the bf16 copy; the affine h = a*x + b is
    folded into the projection weights/biases.
  - Q IS ELIMINATED: S = hx^T kk with hx = a*x (one DVE 4x-mode op) and
    kk = (Wg*a)^T x + w, Wg = (Wq^T Wk)/8 host-side, w = Wk^T bq_eff
    folded as a free per-partition bias into the ACT evacuation of kk.
    The m-row bias term sum_j bq_eff[j] k[j,m] is exactly x^T (a*w).
  - Wp IS FOLDED INTO V: v2 = (Wp Wv * a) x, so AV directly produces the
    projected output; the 65th vT ones-column accumulates the softmax
    denominator. Post per n-tile: ACT evacs the den row, a 1-row PE
    matmul broadcasts it to partitions 0:64, DVE does approx-reciprocal
    + a fused (av*recip + bpp) custom op; the +x residual rides the
    output DMA (accum onto the x-prefilled out buffer).
  - Projections and scores run in 64x128 row-tiled mode: two independent
    64-contraction tiles (SBUF partitions 0:64 / 64:128) execute
    concurrently. Score pairs land in rotating 2-bank PSUM pair-slots.
  - exp of the 16.7M scores is the roofline. Work is split across TWO
    engines by pairs:
      * ScalarE: exact exp, [128,1024] PSUM->SBUF fp8e4 (~1 elem/lane/cyc).
      * VectorE: Schraudolph bit-trick exp in ONE tensor_scalar op:
        int8(11.54*s + 55.5) reinterpreted as fp8e4 == exp(s)*(1 +- 7%).
  - AV runs in fp8 DoubleRow mode: one matmul per pair contracts 256
    into a single [65,512] accumulator. AV for tile nt runs one n-tile
    lagged in half-blocks; the last n-tile drains in quarter-blocks.
  - PSUM: 3 pair-slots (6 banks) + av x2 (2 banks) = 8.
"""

import numpy as np
import ml_dtypes

import concourse.bacc as bacc
import concourse.mybir as mybir
import concourse.dve_ops as dve_ops
from concourse.dve_spec import Spec, Src0, Src1, C0, lower
from concourse.dve_uop import DveOpSpec
from concourse.tile import TileContext
from concourse.bass_utils import run_bass_kernel_spmd


def _get_muladd():
    """out = in0*in1 + s0 as one DVE op (registered via the documented
    OPS-append extension path; sha pinned programmatically)."""
    for op in dve_ops.OPS:
        if op.name == "ATT_MULADD":
            return op
    spec = Spec(body=Src0 * Src1 + C0,
                reference=lambda in0, in1, s0, s1, imm2: in0 * in1 + s0)
    row = dve_ops._CUSTOM_DVE_ROW_BASE + len(dve_ops.OPS)
    shas = {}
    for ver in ("v3", "v4"):
        shas[ver] = DveOpSpec(name="ATT_MULADD", opcode=row,
                              uops=lower(spec, ver=ver), rd1_en=True).sha(ver)
    op = dve_ops.DveOp("ATT_MULADD", spec, subdim=False, uops_sha=shas)
    dve_ops.OPS.append(op)
    dve_ops.CUSTOM_DVE_SPECS[op.name] = spec
    dve_ops._SUB_OPCODE_FOR_NAME[op.name] = row
    return op


MULADD = _get_muladd()

FP = mybir.dt.float32
F16 = mybir.dt.bfloat16
I16 = mybir.dt.int16
F8 = mybir.dt.float8e4
I8 = mybir.dt.int8
B, C, H, W = 8, 64, 64, 64
N = H * W            # 4096
G = 8
NT = 512             # n-tile width
N_NT = N // NT       # 8
NPAIR = 16           # score pairs (of 2x128 m-rows) per n-tile
EPS = 1e-5
COPY = mybir.ActivationFunctionType.Copy
IDENT = mybir.ActivationFunctionType.Identity
EXP = mybir.ActivationFunctionType.Exp
ADD = mybir.AluOpType.add
MUL = mybir.AluOpType.mult
SUB = mybir.AluOpType.subtract

# Schraudolph constants for fp8e4 target (2^3 mantissa scale); scores are
# in [-2.84, 2.84] for this problem so exp fits fp8e4 with no shift and the
# int8 bits stay in [20, 90].
A_SCH = 11.5415603
B_SCH = 55.5

# which pairs each n-tile sends to the DVE trick-exp (rest go to ACT).
# Early n-tiles lighter on DVE (it carries kk/vT evacuations there).
_D7 = {2, 4, 7, 9, 11, 13, 15}
_D8 = {1, 3, 5, 7, 9, 11, 13, 15}
DVE_MAP = {0: {4, 9, 13}, 1: {2, 5, 8, 11, 14}, 2: _D7, 3: _D8,
           4: _D7, 5: _D8, 6: _D8, 7: _D8}

last_run_info = {}


class OneActSetBacc(bacc.Bacc):
    """Force every ACT table load to set 6 (natural_log_exp_and_others:
    exp/ln/square/copy) and drop redundant reloads."""

    NL_EXP_SET = 6

    def insert_act_table_loads(self):
        super().insert_act_table_loads()
        for blk in self.main_func.blocks:
            keep = []
            seen = False
            for ins in blk.instructions:
                if isinstance(ins, mybir.InstLoadActFuncSet):
                    ins.act_func_set_id = self.NL_EXP_SET
                    si = ins.sync_info
                    clean = si is None or (not si.on_wait and not si.on_update)
                    if seen and clean:
                        continue
                    seen = True
                keep.append(ins)
            if len(keep) != len(blk.instructions):
                blk.instructions[:] = keep


def build_program():
    nc = OneActSetBacc()

    x_d = nc.dram_tensor("x", [C, N], FP, kind="ExternalInput")
    # cf32 [128, 141]: 0 bq2(=bq/8) | 1 bpp | 2 gamma2 | 3 beta2 | 4:12 gmask
    #                  | 12:140 gbcast2 (rows 0:8) | 140 eps
    cf32_d = nc.dram_tensor("cf32", [128, 141], FP, kind="ExternalInput")
    # cb16 [128, 512]: 0:128 wq64 | 128:256 wg2 ((Wq^T Wk)/8 tiled 2x2)
    #                  | 256:320 wv2 ((Wp Wv)^T tiled 2x1) | 320:448 wkN2
    #                  (Wk natural [j,c] tiled 2x2)
    cb16_d = nc.dram_tensor("cb16", [128, 512], F16, kind="ExternalInput")
    out_d = nc.dram_tensor("out", [C, N], FP, kind="ExternalOutput")

    with TileContext(nc) as tc:
        with (
            tc.tile_pool(name="const", bufs=1) as const,
            tc.tile_pool(name="big", bufs=1) as big,
            tc.tile_pool(name="epool", bufs=22) as epool,
            tc.tile_pool(name="small", bufs=4) as small,
            tc.tile_pool(name="scr", bufs=2) as scrp,
            tc.tile_pool(name="outp", bufs=3) as outp,
            tc.tile_pool(name="sps", bufs=3, space="PSUM") as sps,
            tc.tile_pool(name="avp", bufs=2, space="PSUM") as avp,
        ):
            # ---------------- constants ----------------
            cf32s = small.tile([128, 141], FP, tag="cf32s")
            cb16s = small.tile([128, 512], F16, tag="cb16s")
            nc.sync.dma_start(out=cf32s[:], in_=cf32_d[:])
            nc.sync.dma_start(out=cb16s[:], in_=cb16_d[:])
            cf32 = const.tile([128, 141], FP, tag="cf32")
            cb16 = const.tile([128, 512], F16, tag="cb16")
            nc.vector.tensor_copy(out=cf32[:], in_=cf32s[:])
            nc.vector.tensor_copy(out=cb16[:], in_=cb16s[:])
            bq2 = cf32[:, 0:1]
            bpp = cf32[0:C, 1:2]
            gamma2 = cf32[:, 2:3]
            beta2 = cf32[:, 3:4]
            gmask = cf32[0:C, 4:12]
            gbcast2 = cf32[0:G, 12:140]
            eps_sb = cf32[:, 140:141]
            wq64 = cb16[:, 0:128]
            wg2 = cb16[:, 128:256]
            wv2 = cb16[:, 256:320]
            wkN2 = cb16[:, 320:448]

            ones_col = const.tile([128, C], F16, tag="ones_col")
            nc.vector.memset(ones_col[:], 1.0)

            # ---------------- x16 load (casting DMA, dup halves) + stats ----
            x16 = big.tile([128, N], F16, tag="x16")
            NCH = 4
            CH = N // NCH
            for j in range(NCH):
                sl = slice(j * CH, (j + 1) * CH)
                nc.gpsimd.dma_start(out=x16[0:C, sl], in_=x_d[:, sl])
            nc.gpsimd.dma_start(out=x16[C:128, :], in_=x_d[:, :])

            sums = small.tile([C, NCH, 2], FP, tag="gn_sums")
            for j in range(NCH):
                sl = slice(j * CH, (j + 1) * CH)
                scr = scrp.tile([C, CH], FP, tag="gn_scr")
                nc.scalar.activation(out=scr[:], in_=x16[0:C, sl],
                                     func=mybir.ActivationFunctionType.Square,
                                     accum_out=sums[:, j, 1:2])
                nc.vector.tensor_reduce(op=ADD, out=sums[:, j, 0:1],
                                        in_=x16[0:C, sl], axis=mybir.AxisListType.X)
            nc.vector.tensor_add(out=sums[:, 0:2, :], in0=sums[:, 0:2, :],
                                 in1=sums[:, 2:4, :])
            nc.vector.tensor_add(out=sums[:, 0, :], in0=sums[:, 0, :],
                                 in1=sums[:, 1, :])

            # PSUM slot rotation: 3 independent [128, 1024] pool tiles (2
            # banks each) so the Tile tracker sequences per-slot, not
            # whole-tile.
            slot_ctr = [0]

            def next_slot():
                slot_ctr[0] += 1
                return sps.tile([128, 1024], FP, tag="sp",
                                name=f"sp_{slot_ctr[0]}")

            # group stats: [G, 2] = gmask.T @ sums  (gmask holds 1/(8N))
            gslot = next_slot()
            nc.tensor.matmul(out=gslot[0:G, 0:2], lhsT=gmask, rhs=sums[:, 0, :])
            gs = small.tile([G, 2], FP, tag="gn_gs")
            nc.vector.tensor_copy(out=gs[:], in_=gslot[0:G, 0:2])
            # nvg = mean^2 - E[x^2]; ln(var+eps) = Ln(-1*nvg + eps)
            nvg = small.tile([G, 1], FP, tag="gn_nvg")
            nc.vector.scalar_tensor_tensor(out=nvg[:], in0=gs[:, 0:1],
                                           scalar=gs[:, 0:1], in1=gs[:, 1:2],
                                           op0=MUL, op1=SUB)
            lnv = small.tile([G, 1], FP, tag="gn_lnv")
            nc.scalar.activation(out=lnv[:], in_=nvg[:],
                                 func=mybir.ActivationFunctionType.Ln,
                                 scale=-1.0, bias=eps_sb[0:G, :])
            rhs2 = small.tile([G, 2], FP, tag="gn_rhs2")
            nc.vector.tensor_copy(out=rhs2[:, 0:1], in_=gs[:, 0:1])
            nc.scalar.activation(out=rhs2[:, 1:2], in_=lnv[:], func=EXP,
                                 scale=-0.5)
            pslot = next_slot()
            nc.tensor.matmul(out=pslot[:, 0:2], lhsT=gbcast2, rhs=rhs2[:])
            a_sb = small.tile([128, 1], FP, tag="gn_a")
            nc.vector.tensor_mul(out=a_sb[:], in0=pslot[:, 1:2], in1=gamma2)
            # negb16 = mean*a - beta  (bf16; = -b)
            negb16 = small.tile([128, 1], F16, tag="gn_negb")
            nc.vector.scalar_tensor_tensor(out=negb16[:], in0=pslot[:, 0:1],
                                           scalar=a_sb[:], in1=beta2,
                                           op0=MUL, op1=SUB)
            wg_eff = const.tile([128, 128], F16, tag="wg_eff")
            wv2_eff = const.tile([128, C], F16, tag="wv2_eff")
            nc.vector.tensor_scalar_mul(out=wg_eff[:], in0=wg2, scalar1=a_sb[:])
            nc.vector.tensor_scalar_mul(out=wv2_eff[:], in0=wv2, scalar1=a_sb[:])
            # biases: bq_eff = bq/8 - (Wq(-b))/8 ; bpp_eff = bpp - WpWv(-b)
            bslot = next_slot()
            nc.tensor.matmul(out=bslot[:, 0:1], lhsT=wq64[0:C, :],
                             rhs=negb16[0:C, :])
            nc.tensor.matmul(out=bslot[0:C, 1:2], lhsT=wv2[0:C, :],
                             rhs=negb16[0:C, :])
            bq16 = small.tile([128, 1], F16, tag="bq16")
            bpp_eff = small.tile([C, 1], FP, tag="bpp_eff")
            nc.vector.tensor_sub(out=bq16[:], in0=bq2, in1=bslot[:, 0:1])
            nc.vector.tensor_sub(out=bpp_eff[:], in0=bpp, in1=bslot[0:C, 1:2])
            # w = Wk^T bq_eff (per-partition bias folded into kk evac)
            wslot = next_slot()
            nc.tensor.matmul(out=wslot[:, 0:1], lhsT=wkN2[0:C, :],
                             rhs=bq16[0:C, :])
            w_sb = small.tile([128, 1], FP, tag="w_sb")
            nc.vector.tensor_copy(out=w_sb[:], in_=wslot[:, 0:1])

            # hx = a * x (scores lhsT; carries the m-side GroupNorm scale)
            hx = big.tile([128, N], F16, tag="hx")
            for j in range(NCH):
                sl = slice(j * CH, (j + 1) * CH)
                nc.vector.tensor_scalar_mul(out=hx[:, sl], in0=x16[:, sl],
                                            scalar1=a_sb[:])

            # ---------------- projections (row-tiled pairs) ----------------
            kk2x = big.tile([128, N], F16, tag="kk2x")
            vT = big.tile([128, N // 128, 80], F8, tag="vT")  # 80-byte chunk stride (DoubleRow needs %16==0)
            nc.vector.memset(vT[:, :, C:C + 1], 1.0)

            def emit_kproj(r):
                lo = slice(1024 * r, 1024 * r + 512)
                hi = slice(1024 * r + 512, 1024 * r + 1024)
                s = next_slot()
                nc.tensor.matmul(out=s[:, 0:512], lhsT=wg_eff[0:C, :],
                                 rhs=x16[0:C, lo])
                nc.tensor.matmul(out=s[:, 512:1024], lhsT=wg_eff[C:128, :],
                                 rhs=x16[C:128, hi])
                if r == 0:
                    # split evac so scores pair 0 (kk cols 0:512) starts early
                    nc.scalar.activation(out=kk2x[:, 0:512],
                                         in_=s[:, 0:512], func=IDENT,
                                         bias=w_sb[:])
                    nc.scalar.activation(out=kk2x[:, 512:1024],
                                         in_=s[:, 512:1024], func=IDENT,
                                         bias=w_sb[:])
                else:
                    nc.scalar.activation(out=kk2x[:, 1024 * r:1024 * (r + 1)],
                                         in_=s[:, 0:1024], func=IDENT,
                                         bias=w_sb[:])

            def emit_vproj(g):
                # chunks 8g..8g+7: T0 evens into bank cols 0:256, T8 odds 512:768
                s = next_slot()
                for t in range(4):
                    ce = 8 * g + 2 * t
                    nc.tensor.matmul(out=s[0:128, 64 * t:64 * t + 64],
                                     lhsT=x16[0:C, ce * 128:(ce + 1) * 128],
                                     rhs=wv2_eff[0:C, :])
                    nc.tensor.matmul(out=s[0:128, 512 + 64 * t:512 + 64 * t + 64],
                                     lhsT=x16[C:128, (ce + 1) * 128:(ce + 2) * 128],
                                     rhs=wv2_eff[C:128, :])
                dst = vT[:, 8 * g:8 * g + 8, 0:C].rearrange(
                    "p (t q) c -> p q t c", q=2)
                src = s[:, 0:1024].rearrange("p (q x) -> p q x", q=2)[
                    :, :, 0:256].rearrange("p q (t c) -> p q t c", t=4)
                nc.vector.tensor_copy(out=dst, in_=src)

            # ---------------- attention loop ----------------
            def emit_scores(nt, p):
                nsl = slice(nt * NT, (nt + 1) * NT)
                s = next_slot()
                c0 = 2 * p
                nc.tensor.matmul(out=s[:, 0:512],
                                 lhsT=hx[0:C, c0 * 128:(c0 + 1) * 128],
                                 rhs=kk2x[0:C, nsl], skip_group_check=True)
                nc.tensor.matmul(out=s[:, 512:1024],
                                 lhsT=hx[C:128, (c0 + 1) * 128:(c0 + 2) * 128],
                                 rhs=kk2x[C:128, nsl], skip_group_check=True)
                return s

            def emit_consumer(nt, p, s, dve):
                e = epool.tile([128, 1024], F8, tag="e", name=f"e_{nt}_{p}")
                if dve:
                    nc.vector.tensor_scalar(
                        out=e[:].bitcast(I8), in0=s[:, 0:1024],
                        scalar1=A_SCH, scalar2=B_SCH, op0=MUL, op1=ADD)
                else:
                    nc.scalar.activation(out=e[:], in_=s[:, 0:1024], func=EXP)
                return e

            def emit_av(av, e, p, first, last):
                # fp8 DoubleRow: both 128-row chunks of the pair in one MM
                # (contraction 256 over 128 partitions x 2 interleaved).
                nc.tensor.matmul(out=av[0:C + 1, :],
                                 lhsT=vT[:, 2 * p:2 * p + 2, 0:C + 1],
                                 rhs=e[:].rearrange("p (q n) -> p q n", q=2),
                                 perf_mode=mybir.MatmulPerfMode.DoubleRow,
                                 start=first, stop=last,
                                 skip_group_check=True)

            def emit_post(nt, av):
                # av rows 0:64 already hold the PROJECTED output (Wp folded
                # into v2); row 64 is the softmax denominator. Broadcast den
                # to partitions 0:64 via a 1-row matmul, then
                # out = av*recip(den) + bpp; +x rides the output accum-DMA.
                nsl = slice(nt * NT, (nt + 1) * NT)
                den16 = outp.tile([128, NT], F16, tag="den16")
                nc.scalar.activation(out=den16[C:C + 1, :],
                                     in_=av[C:C + 1, :], func=COPY)
                s = next_slot()
                nc.tensor.matmul(out=s[0:C, 0:512],
                                 lhsT=ones_col[C:C + 1, :],
                                 rhs=den16[C:C + 1, :], skip_group_check=True)
                dbc = outp.tile([C, NT], FP, tag="dbc")
                nc.vector.reciprocal_approx_fast(out=dbc[:], in_=s[0:C, 0:512])
                o_sb = outp.tile([C, NT], FP, tag="o_sb")
                nc.vector._custom_dve(MULADD, out=o_sb[:], in0=av[0:C, :],
                                      in1=dbc[:], s0=bpp_eff[:])
                nc.vector.tensor_add(out=o_sb[:], in0=o_sb[:],
                                     in1=x16[0:C, nsl])
                nc.sync.dma_start(out=out_d[:, nsl], in_=o_sb[:])

            # startup cascade + steady loop. AV for n-tile nt runs one tile
            # LAGGED, in two half-blocks of 8 DoubleRow MMs, so the PE mode
            # (64x128 scores vs 128x128 DoubleRow) switches only ~4x per nt.
            emit_kproj(0)
            emit_vproj(0)

            CASCADE = {2: [("k", 1)], 4: [("v", 1)],
                       6: [("k", 2)], 8: [("v", 2)],
                       10: [("k", 3)], 12: [("v", 3)]}
            e_tiles = {}
            av_tiles = {}

            def emit_av_block(nt, lo, hi):
                av = av_tiles[nt]
                for p in range(lo, hi):
                    emit_av(av, e_tiles.pop((nt, p)), p,
                            first=(p == 0), last=(p == NPAIR - 1))

            LAST = N_NT - 1
            for nt in range(N_NT):
                dve_set = DVE_MAP[nt]
                av_tiles[nt] = avp.tile([128, NT], FP, tag="av",
                                        name=f"av_{nt}")
                pend = {}
                for p in range(NPAIR):
                    if nt == 0:
                        for kind, r in CASCADE.get(p, []):
                            (emit_kproj if kind == "k" else emit_vproj)(r)
                    pend[p] = emit_scores(nt, p)
                    if p >= 1:
                        e_tiles[(nt, p - 1)] = emit_consumer(
                            nt, p - 1, pend.pop(p - 1), (p - 1) in dve_set)
                    if nt > 0 and p == 7:
                        emit_av_block(nt - 1, 0, 8)
                    if nt > 0 and p == 15:
                        emit_av_block(nt - 1, 8, 16)
                    # last n-tile: drain AV in quarter blocks to shrink the
                    # tail after the final exp
                    if nt == LAST and p in (5, 9, 13):
                        emit_av_block(nt, 4 * ((p - 5) // 4), 4 * ((p - 5) // 4) + 4)
                e_tiles[(nt, NPAIR - 1)] = emit_consumer(
                    nt, NPAIR - 1, pend.pop(NPAIR - 1),
                    (NPAIR - 1) in dve_set)
                if nt > 0:
                    emit_post(nt - 1, av_tiles.pop(nt - 1))
            emit_av_block(LAST, 12, 16)
            emit_post(LAST, av_tiles.pop(LAST))

    nc.finalize()
    return nc


def make_consts(Wq, bq, Wk, Wv, bv, Wp, bp, gn_w, gn_b):
    f32 = np.float32
    gmask = np.zeros((C, G), f32)
    gbcast2 = np.zeros((G, 128), f32)
    for g in range(G):
        gmask[g * 8:(g + 1) * 8, g] = 1.0 / (8.0 * N)
        gbcast2[g, g * 8:(g + 1) * 8] = 1.0
        gbcast2[g, C + g * 8:C + (g + 1) * 8] = 1.0
    Wq_ = np.asarray(Wq, f32)
    Wk_ = np.asarray(Wk, f32)
    Wv_ = np.asarray(Wv, f32)
    Wp_ = np.asarray(Wp, f32)
    WqT = Wq_.T
    cf32 = np.zeros((128, 141), f32)
    cf32[:, 0] = np.tile(np.asarray(bq, f32) / 8.0, 2)
    cf32[0:C, 1] = np.asarray(bp, f32) + Wp_ @ np.asarray(bv, f32)
    cf32[:, 2] = np.tile(np.asarray(gn_w, f32), 2)
    cf32[:, 3] = np.tile(np.asarray(gn_b, f32), 2)
    cf32[0:C, 4:12] = gmask
    cf32[0:G, 12:140] = gbcast2
    cf32[:, 140] = EPS
    cb16 = np.zeros((128, 512), f32)
    cb16[:, 0:128] = np.tile(WqT, (2, 2)) / 8.0
    cb16[:, 128:256] = np.tile((Wq_.T @ Wk_) / 8.0, (2, 2))
    cb16[:, 256:320] = np.tile((Wp_ @ Wv_).T, (2, 1))
    cb16[:, 320:448] = np.tile(Wk_, (2, 2))
    return {
        "cf32": np.ascontiguousarray(cf32),
        "cb16": np.ascontiguousarray(cb16.astype(ml_dtypes.bfloat16)),
    }


_cached = {}


def _install_trace_hook():
    import sys, types
    import antenv
    if "antenv.axon_hooks" in sys.modules:
        return
    mod = types.ModuleType("antenv.axon_hooks")
    holder = {"hook": None}
    mod.set_axon_ntff_profile_hook = lambda h: holder.__setitem__("hook", h)
    mod.get_axon_ntff_profile_hook = lambda: holder["hook"]
    sys.modules["antenv.axon_hooks"] = mod
    antenv.axon_hooks = mod
    from trn_agent_boot.trn_boot import _ntff_profile_via_ctypes
    mod.set_axon_ntff_profile_hook(_ntff_profile_via_ctypes("/opt/axon/libaxon_pjrt.so"))
    import concourse.bass_utils as bu
    bu.upload_artifacts = lambda tmpdir: tmpdir


def kernel(x, gn_w, gn_b, Wq, bq, Wk, bk, Wv, bv, Wp, bp, _trace=False):
    x = np.ascontiguousarray(np.asarray(x, np.float32)).reshape(B, C, N)
    consts = make_consts(Wq, bq, Wk, Wv, bv, Wp, bp, gn_w, gn_b)

    if _trace:
        _install_trace_hook()

    if "nc" not in _cached:
        _cached["nc"] = build_program()
    nc = _cached["nc"]

    in_maps = [dict(consts, x=np.ascontiguousarray(x[i])) for i in range(B)]
    res = run_bass_kernel_spmd(nc, in_maps, core_ids=list(range(B)), trace=_trace)
    last_run_info["exec_time_ns"] = res.exec_time_ns
    last_run_info["mean_exec_time_ns"] = res.mean_exec_time_ns
    out = np.stack([res.results[i]["out"] for i in range(B)], axis=0)
    return out.reshape(B, C, H, W)
